# revision 18
# baseline (speedup 1.0000x reference)
"""Trainium2 Bass kernel for nn_BallPredictorGNN.

The reference model is a 2-layer GAT over (N=20000, E=640000) followed by an
MLP applied to the LAST node only ("ball") — the output is a single [2] vector.
Only the ball's 2-hop dependency cone matters:

  layer 2 aggregates at the ball node only            (~25 in-edges)
  layer 1 aggregates at the ball's in-neighbours S2   (~25 nodes, ~800 edges)
  x @ W1 is needed for the sources of those edges     (~800 edges)

Host side (pure data routing): extract the cone and lay layer-1 edges out on a
[128 partitions x K chunks] grid, where each partition serves one destination
node (high-degree destinations get several partitions, merged at the end by a
single one-hot matmul).  The source features are replicated per edge-slot into
the xT operand, so the projection matmul directly produces per-edge rows
[ad | as | h] = x_src @ [W1Ad | W1As | W1] in the right partition — no
gather, no DRAM round-trip, no indirect DMA anywhere.

Device side (all FLOPs): per chunk, one TensorE matmul projects 128 edges;
VectorE computes e = as + ad + mask, leaky-relu, and accumulates
msg += h * exp(e) and den += exp(e) along the free axis
(alpha = exp(e)/sum exp(e) folded as  out = (sum exp(e)*h_src) / sum exp(e);
masked/padded slots contribute exp(-1e30) = 0).  Layer 2 (ball only) runs
fully on-chip with one-hot matmuls against the SBUF-resident projection.

The same program is replicated SPMD on all 8 NeuronCores (the cone is tiny, so
replication beats sharding + collectives); core 0's output is returned.
"""

import numpy as np

P = 128
_CACHE = {}


def _ceil(a, b):
    return -(-a // b)


def _pad_rows(a, n, fill=0):
    out = np.full((n,) + a.shape[1:], fill, a.dtype)
    out[: len(a)] = a
    return out


class _Packer:
    """Pack many small [p, w] operands into one [128, W] array, column-wise."""

    def __init__(self, dtype=np.float32):
        self.cols = []
        self.pos = 0
        self.slots = {}
        self.dtype = dtype

    def add(self, name, arr):
        p, w = arr.shape
        full = np.zeros((P, w), self.dtype)
        full[:p] = arr
        self.cols.append(full)
        self.slots[name] = (self.pos, self.pos + w)
        self.pos += w

    def finish(self):
        return np.ascontiguousarray(np.concatenate(self.cols, axis=1))


NEG = np.float32(-1e30)


def _host_preprocess(inputs):
    x = np.asarray(inputs["x"], dtype=np.float32)
    ei = np.asarray(inputs["edge_index"]).astype(np.int64)
    N = x.shape[0]
    F = x.shape[1]
    ball = N - 1
    src, dst = ei[0], ei[1]

    # ---- layer-2 cone: edges into the ball (+ self loop) --------------------
    sel2 = dst == ball
    e2s = np.concatenate([src[sel2], [ball]])
    uniq = np.unique(e2s)
    S2 = np.concatenate([[ball], uniq[uniq != ball]]).astype(np.int64)
    m2 = len(S2)
    assert m2 <= 127, f"ball in-neighbourhood too large for one dst block: {m2}"

    # ---- layer-1 edge grid: [partition, chunk] ------------------------------
    in_S2 = np.zeros(N, dtype=bool)
    in_S2[S2] = True
    sel1 = in_S2[dst]
    l1s, l1d = src[sel1], dst[sel1]  # self loops handled separately

    # per-destination source lists (excluding the self loop)
    loc2 = np.full(N, -1, dtype=np.int64)
    loc2[S2] = np.arange(m2)
    by_dst = [[] for _ in range(m2)]
    for s, d in zip(l1s, loc2[l1d]):
        by_dst[d].append(s)

    # choose K (chunks) so all partition groups fit in 128 partitions
    K = 2
    while sum(max(1, _ceil(len(g), K - 1)) for g in by_dst) > P:
        K += 1
    K = max(K, 3)

    grid_src = np.zeros((P, K), dtype=np.int64)  # source node per slot
    grid_valid = np.zeros((P, K), dtype=bool)
    slotmap = np.full(P, P - 1, dtype=np.int64)  # partition -> dst slot
    p = 0
    for sidx in range(m2):
        g = by_dst[sidx]
        v = S2[sidx]
        nparts = max(1, _ceil(len(g), K - 1))
        for gi in range(nparts):
            grid_src[p, 0] = v  # self loop (duplicates masked)
            grid_valid[p, 0] = gi == 0
            chunk_edges = g[gi * (K - 1) : (gi + 1) * (K - 1)]
            for j, s in enumerate(chunk_edges):
                grid_src[p, 1 + j] = s
                grid_valid[p, 1 + j] = True
            slotmap[p] = sidx
            p += 1
    assert p <= P

    # xT: [F, K*128] with column k*128+q = x[grid_src[q, k]] (bf16).
    # Chunk-0 columns keep their features even when masked: secondary
    # partitions read a_dst[dst] from their (duplicate) self-loop row.
    zero_slots = ~grid_valid
    zero_slots[:, 0] = False
    xg = x[grid_src.T.reshape(-1)]  # [K*128, F]
    xg[zero_slots.T.reshape(-1)] = 0
    import ml_dtypes

    xT = np.ascontiguousarray(xg.T.astype(ml_dtypes.bfloat16))  # [F, K*128]

    admask = np.where(grid_valid, np.float32(0), NEG).astype(np.float32)  # [P,K]
    pmapcol = slotmap[:, None].astype(np.float32)  # [P,1]

    # ---- layer-2 index rows -------------------------------------------------
    s2_loc = loc2[e2s]  # all < m2
    n2 = len(s2_loc)
    T2 = _ceil(n2, P)
    assert T2 == 1, f"layer-2 edge count exceeds one tile: {n2}"
    n2p = T2 * P

    def row(a, n_pad, fill, dt):
        return _pad_rows(a.astype(dt), n_pad, fill)[None, :]

    # ---- dense operands -----------------------------------------------------
    W1 = np.asarray(inputs["W1"], np.float32)  # [F, 4*64]
    a_src1 = np.asarray(inputs["a_src1"], np.float32)  # [4, 64]
    a_dst1 = np.asarray(inputs["a_dst1"], np.float32)
    H1, C = a_src1.shape
    D1 = H1 * C
    ablk = np.zeros((D1, 2 * H1), np.float32)  # [256, 8] = [Ad | As]
    for h in range(H1):
        ablk[h * C : (h + 1) * C, h] = a_dst1[h]
        ablk[h * C : (h + 1) * C, H1 + h] = a_src1[h]

    W2 = np.asarray(inputs["W2"], np.float32)  # [256, 64]
    a2 = np.stack(
        [np.asarray(inputs["a_dst2"], np.float32)[0],
         np.asarray(inputs["a_src2"], np.float32)[0]],
        axis=1,
    )  # [64, 2] = [a_dst | a_src]

    pkw = _Packer()
    W1T = np.ascontiguousarray(W1.T)
    for k in range(D1 // P):
        pkw.add(f"w1T{k}", W1T[k * P : (k + 1) * P])
        pkw.add(f"ablk{k}", ablk[k * P : (k + 1) * P])

    pka = _Packer()
    pka.add("w1", W1)
    pka.add("admask", admask)
    pka.add("pmapcol", pmapcol)

    pkb = _Packer()
    for k in range(D1 // P):
        pkb.add(f"w2_{k}", W2[k * P : (k + 1) * P])
    pkb.add("b1bc", np.broadcast_to(np.asarray(inputs["b1"], np.float32), (P, D1)))
    pkb.add("w2T", np.ascontiguousarray(W2.T))
    pkb.add("a2", a2)
    pkb.add("b2col", np.asarray(inputs["b2"], np.float32)[:, None])
    pkb.add("fc1w", np.ascontiguousarray(np.asarray(inputs["fc1_w"], np.float32)))
    pkb.add("fc1b", np.asarray(inputs["fc1_b"], np.float32)[:, None])
    pkb.add("fc2w", np.ascontiguousarray(np.asarray(inputs["fc2_w"], np.float32)))
    pkb.add("fc2b", np.asarray(inputs["fc2_b"], np.float32)[:, None])
    pkb.add("dstrel2", np.ascontiguousarray(
        _pad_rows(np.zeros(n2, np.float32), n2p, 1)[:, None]))
    pkb.add("l2rep", np.broadcast_to(np.concatenate(
        [row(s2_loc, n2p, 0, np.float32),
         row(np.zeros(n2), n2p, 1, np.float32)], axis=1), (P, 2 * n2p)))

    feed = {"xT": xT, "packw": pkw.finish(), "packa": pka.finish(),
            "packb": pkb.finish()}
    dims = dict(
        F=F, H1=H1, C=C, K=K, m2=m2, T2=T2,
        slots_w=tuple(sorted(pkw.slots.items())),
        slots_a=tuple(sorted(pka.slots.items())),
        slots_b=tuple(sorted(pkb.slots.items())),
    )
    return feed, dims


def _build(dims):
    from concourse import bacc, bass, mybir, tile
    from concourse.masks import make_identity

    F = dims["F"]          # 128 input features
    H1 = dims["H1"]        # 4 heads, layer 1
    C = dims["C"]          # 64 channels per head
    D1 = H1 * C            # 256
    G1W = 2 * H1 + D1      # 264 = [ad(4) | as(4) | h(256)]
    G2W = 2 + C            # 66  = [ad2 | as2 | h2p]
    K = dims["K"]          # layer-1 chunks (edge slots per partition)
    KCH = D1 // P          # 2 contraction chunks over 256
    slots_w = dict(dims["slots_w"])
    slots_a = dict(dims["slots_a"])
    slots_b = dict(dims["slots_b"])
    WW = max(b for _, b in slots_w.values())
    WA = max(b for _, b in slots_a.values())
    WB = max(b for _, b in slots_b.values())
    f32 = mybir.dt.float32
    bf16 = mybir.dt.bfloat16

    nc = bacc.Bacc("TRN2", target_bir_lowering=False, debug=False)

    xT_d = nc.declare_dram_parameter("xT", [F, K * P], bf16, isOutput=False)
    pw_d = nc.declare_dram_parameter("packw", [P, WW], f32, isOutput=False)
    pa_d = nc.declare_dram_parameter("packa", [P, WA], f32, isOutput=False)
    pb_d = nc.declare_dram_parameter("packb", [P, WB], f32, isOutput=False)
    out_d = nc.declare_dram_parameter("out", [2, 1], f32, isOutput=True)

    EQ = mybir.AluOpType.is_equal
    MAX = mybir.AluOpType.max
    ADD = mybir.AluOpType.add
    MUL = mybir.AluOpType.mult
    Copy = mybir.ActivationFunctionType.Copy
    Exp = mybir.ActivationFunctionType.Exp
    Relu = mybir.ActivationFunctionType.Relu

    with tile.TileContext(nc) as tc:
        with (
            tc.tile_pool(name="const", bufs=1) as cp,
            tc.tile_pool(name="work", bufs=3) as wp,
            tc.tile_pool(name="fin", bufs=1) as fp,
            tc.tile_pool(name="psum", bufs=2, space="PSUM") as pp,
            tc.tile_pool(name="pgp", bufs=3, space="PSUM") as pgp,
            tc.tile_pool(name="acc", bufs=1, space="PSUM") as ap_,
        ):
            # ---------------- inputs into SBUF -----------------------------
            pkw_s = cp.tile([P, WW], f32)
            nc.sync.dma_start(pkw_s[:], pw_d[:])
            pka_s = cp.tile([P, WA], f32)
            nc.sync.dma_start(pka_s[:], pa_d[:])
            xT_s = cp.tile([F, K * P], bf16)
            nc.sync.dma_start(xT_s[:], xT_d[:])
            pkb_s = cp.tile([P, WB], f32)
            nc.gpsimd.dma_start(pkb_s[:], pb_d[:])

            def fsl(name, rows=P):
                if name in slots_w:
                    a, b = slots_w[name]
                    return pkw_s[:rows, a:b]
                if name in slots_a:
                    a, b = slots_a[name]
                    return pka_s[:rows, a:b]
                a, b = slots_b[name]
                return pkb_s[:rows, a:b]

            ident = cp.tile([P, P], f32)
            make_identity(nc, ident[:])
            iota_f = cp.tile([P, P], f32)
            nc.gpsimd.iota(
                iota_f[:], pattern=[[1, P]], base=0, channel_multiplier=0,
                allow_small_or_imprecise_dtypes=True,
            )
            iota_c = cp.tile([P, 1], f32)
            nc.gpsimd.iota(
                iota_c[:], pattern=[[0, 1]], base=0, channel_multiplier=1,
                allow_small_or_imprecise_dtypes=True,
            )

            # ---------------- W1 @ [Ad | As]  (K = 256, 2 chunks) ----------
            pwa = pp.tile([F, 2 * H1], f32, tag="mm")
            for k in range(KCH):
                nc.tensor.matmul(
                    out=pwa[:], lhsT=fsl(f"w1T{k}"), rhs=fsl(f"ablk{k}"),
                    start=(k == 0), stop=(k == KCH - 1),
                )
            rhs1 = cp.tile([F, G1W], bf16)
            nc.vector.tensor_copy(rhs1[:, : 2 * H1], pwa[:])
            nc.scalar.copy(rhs1[:, 2 * H1 :], fsl("w1"))

            # ---------------- layer-1 edge chunks --------------------------
            # chunk k: project 128 edge slots, form [h*exp(e) | exp(e)], and
            # accumulate + merge partition groups in PSUM via the one-hot
            # partition->slot matmul:  agg1 += pmap @ [msg | exp(e)]
            pmap = fp.tile([P, P], bf16)
            nc.vector.tensor_scalar(
                pmap[:], iota_f[:], fsl("pmapcol")[:, 0:1], None, EQ
            )
            agg1 = ap_.tile([P, D1 + H1], f32, tag="agg1")
            ad_part = fp.tile([P, H1], f32)
            admix = fp.tile([P, K * H1], f32)
            for k in range(K):
                pg = pgp.tile([P, G1W], f32, tag="pg")
                nc.tensor.matmul(
                    out=pg[:], lhsT=xT_s[:, k * P : (k + 1) * P],
                    rhs=rhs1[:], start=True, stop=True,
                )
                if k == 0:
                    # a_dst per partition from the self-loop rows, then fold
                    # in the validity mask for every chunk at once
                    nc.vector.tensor_copy(ad_part[:], pg[:, :H1])
                    nc.vector.tensor_tensor(
                        out=admix[:].rearrange("p (k h) -> p k h", h=H1),
                        in0=ad_part[:].rearrange("p (o h) -> p o h", o=1)
                        .to_broadcast([P, K, H1]),
                        in1=fsl("admask")[:].rearrange("p (k o) -> p k o", o=1)
                        .to_broadcast([P, K, H1]),
                        op=ADD,
                    )
                e = wp.tile([P, H1], f32, tag="e")
                nc.vector.tensor_tensor(
                    out=e[:], in0=pg[:, H1 : 2 * H1],
                    in1=admix[:, k * H1 : (k + 1) * H1], op=ADD,
                )
                es = wp.tile([P, H1], f32, tag="es")
                nc.gpsimd.tensor_scalar_mul(es[:], e[:], 0.2)
                el = wp.tile([P, H1], f32, tag="el")
                nc.vector.tensor_tensor(out=el[:], in0=e[:], in1=es[:], op=MAX)
                pe = wp.tile([P, H1], f32, tag="pe")
                nc.scalar.activation(pe[:], el[:], Exp)
                msg = wp.tile([P, D1 + H1], bf16, tag="msg")
                nc.vector.tensor_tensor(
                    out=msg[:, :D1].rearrange("p (h c) -> p h c", c=C),
                    in0=pg[:, 2 * H1 :].rearrange("p (h c) -> p h c", c=C),
                    in1=pe[:].rearrange("p (h o) -> p h o", o=1).to_broadcast(
                        [P, H1, C]
                    ),
                    op=MUL,
                )
                nc.scalar.activation(msg[:, D1:], pe[:], Copy)
                nc.tensor.matmul(
                    out=agg1[:], lhsT=pmap[:], rhs=msg[:],
                    start=(k == 0), stop=(k == K - 1),
                )

            # layer-2 rhs [W2A2 | W2] per K-chunk + one-hots (no h1r dep)
            rhs2 = []
            for k in range(KCH):
                pwa2 = pp.tile([P, 2], f32, tag="mm")
                nc.tensor.matmul(
                    out=pwa2[:], lhsT=fsl("w2T", C)[:, k * P : (k + 1) * P],
                    rhs=fsl("a2", C), start=True, stop=True,
                )
                rhs2_k = cp.tile([P, G2W], f32, name=f"rhs2_{k}")
                nc.vector.tensor_copy(rhs2_k[:, :2], pwa2[:])
                nc.scalar.copy(rhs2_k[:, 2:], fsl(f"w2_{k}"))
                rhs2.append(rhs2_k)
            # layer-2 one-hots from the replicated [src2 | dstrel2] rows
            l2r = fsl("l2rep")
            st2 = fp.tile([P, P], bf16)
            nc.vector.tensor_scalar(st2[:], l2r[:, :P], iota_c[:, 0:1], None, EQ)
            em2t = fp.tile([P, P], f32)
            nc.vector.tensor_scalar(em2t[:], l2r[:, P:], iota_c[:, 0:1], None, EQ)

            # ---------------- layer-1 finalize: h1r = relu(num/den + b1) ---
            den1 = fp.tile([P, H1], f32)
            nc.vector.tensor_scalar_add(den1[:], agg1[:, D1:], 1e-16)
            rec1 = fp.tile([P, H1], f32)
            nc.vector.reciprocal(rec1[:], den1[:])
            h1t = fp.tile([P, D1], f32)
            nc.vector.tensor_tensor(
                out=h1t[:].rearrange("p (h c) -> p h c", c=C),
                in0=agg1[:, :D1].rearrange("p (h c) -> p h c", c=C),
                in1=rec1[:].rearrange("p (h o) -> p h o", o=1).to_broadcast(
                    [P, H1, C]
                ),
                op=MUL,
            )
            h1b = fp.tile([P, D1], f32)
            nc.vector.tensor_add(h1b[:], h1t[:], fsl("b1bc"))
            h1r = fp.tile([P, D1], f32)
            nc.scalar.activation(h1r[:], h1b[:], Relu)

            # ---------------- layer-2 projection: G2 = [ad2|as2|h2p] -------
            pg2 = ap_.tile([P, G2W], f32, tag="pg2")
            for k in range(KCH):
                ptr = pp.tile([P, P], f32, tag="mm")
                nc.tensor.transpose(
                    out=ptr[:], in_=h1r[:, k * P : (k + 1) * P], identity=ident[:]
                )
                h1rT_k = wp.tile([P, P], f32, tag=f"h1rTk{k}")
                nc.vector.tensor_copy(h1rT_k[:], ptr[:])
                nc.tensor.matmul(
                    out=pg2[:], lhsT=h1rT_k[:], rhs=rhs2[k][:],
                    start=(k == 0), stop=(k == KCH - 1),
                )
            g2sb = fp.tile([P, G2W], bf16)
            nc.scalar.copy(g2sb[:], pg2[:])
            g2ad = fp.tile([P, 1], f32)
            nc.vector.tensor_copy(g2ad[:], pg2[:, 0:1])

            # ------- layer-2 edge aggregation: fully on-chip (ball only) ---
            gs2_p = pp.tile([P, G2W], f32, tag="mm")
            nc.tensor.matmul(out=gs2_p[:], lhsT=st2[:], rhs=g2sb[:],
                             start=True, stop=False, skip_group_check=True)
            # accumulate the a_dst[ball] expansion straight onto the as2
            # column: gs2_p[:, 1] becomes e2 = as2[src] + ad2[dst]
            nc.tensor.matmul(out=gs2_p[:, 1:2], lhsT=em2t[:], rhs=g2ad[:],
                             start=False, stop=True, skip_group_check=True)
            es2 = fp.tile([P, 1], f32)
            nc.vector.tensor_scalar_mul(es2[:], gs2_p[:, 1:2], 0.2)
            el2 = fp.tile([P, 1], f32)
            nc.vector.tensor_tensor(
                out=el2[:], in0=gs2_p[:, 1:2], in1=es2[:], op=MAX
            )
            pe2 = fp.tile([P, 1], f32)
            nc.scalar.activation(pe2[:], el2[:], Exp)
            rhs2t = fp.tile([P, C + 1], bf16)
            nc.vector.tensor_tensor(
                out=rhs2t[:, :C], in0=gs2_p[:, 2:],
                in1=pe2[:].to_broadcast([P, C]), op=MUL,
            )
            nc.scalar.activation(rhs2t[:, C:], pe2[:], Copy)
            em2 = fp.tile([P, P], bf16)
            nc.vector.tensor_scalar(
                em2[:], iota_f[:], fsl("dstrel2")[:, 0:1], None, EQ
            )
            agg2 = ap_.tile([P, C + 1], f32, tag="agg2")
            nc.tensor.matmul(out=agg2[:], lhsT=em2[:], rhs=rhs2t[:],
                             start=True, stop=True)

            # ---------------- ball finalize + MLP --------------------------
            den2 = fp.tile([1, 1], f32)
            nc.vector.tensor_scalar_add(den2[:], agg2[0:1, C : C + 1], 1e-16)
            rec2 = fp.tile([1, 1], f32)
            nc.vector.reciprocal(rec2[:], den2[:])
            bf = fp.tile([1, C], f32)
            nc.scalar.activation(bf[:], agg2[0:1, :C], Copy, scale=rec2[:, 0:1])
            ptb = pp.tile([C, 1], f32, tag="mm")
            nc.tensor.transpose(out=ptb[:], in_=bf[:], identity=ident[0:1, 0:1])
            bfr = fp.tile([C, 1], f32)
            nc.scalar.activation(bfr[:], ptb[:], Relu, bias=fsl("b2col", C))

            pz = pp.tile([C // 2, 1], f32, tag="mm")
            nc.tensor.matmul(out=pz[:], lhsT=fsl("fc1w", C), rhs=bfr[:],
                             start=True, stop=True)
            zr = fp.tile([C // 2, 1], f32)
            nc.scalar.activation(zr[:], pz[:], Relu, bias=fsl("fc1b", C // 2))

            po = pp.tile([2, 1], f32, tag="mm")
            nc.tensor.matmul(out=po[:], lhsT=fsl("fc2w", C // 2), rhs=zr[:],
                             start=True, stop=True)
            osb = fp.tile([2, 1], f32)
            nc.vector.tensor_add(osb[:], po[:], fsl("fc2b", 2))
            nc.sync.dma_start(out_d[:], osb[:])

    nc.compile()
    return nc


def kernel(**inputs):
    from concourse.bass_utils import run_bass_kernel_spmd

    feed, dims = _host_preprocess(inputs)
    key = (dims["K"], dims["m2"], dims["T2"])
    if key not in _CACHE:
        _CACHE[key] = _build(dims)
    nc = _CACHE[key]

    n_cores = 8
    in_maps = [dict(feed) for _ in range(n_cores)]
    res = run_bass_kernel_spmd(nc, in_maps, core_ids=list(range(n_cores)))
    out = np.asarray(res.results[0]["out"], dtype=np.float32).reshape(2)
    return out


# revision 20
# speedup vs baseline: 1.0203x; 1.0203x over previous
"""Trainium2 Bass kernel for nn_BallPredictorGNN.

The reference model is a 2-layer GAT over (N=20000, E=640000) followed by an
MLP applied to the LAST node only ("ball") — the output is a single [2] vector.
Only the ball's 2-hop dependency cone matters:

  layer 2 aggregates at the ball node only            (~25 in-edges)
  layer 1 aggregates at the ball's in-neighbours S2   (~25 nodes, ~800 edges)
  x @ W1 is needed for the sources of those edges     (~800 edges)

Host side (pure data routing): extract the cone and lay layer-1 edges out on a
[128 partitions x K chunks] grid, where each partition serves one destination
node (high-degree destinations get several partitions, merged at the end by a
single one-hot matmul).  The source features are replicated per edge-slot into
the xT operand, so the projection matmul directly produces per-edge rows
[ad | as | h] = x_src @ [W1Ad | W1As | W1] in the right partition — no
gather, no DRAM round-trip, no indirect DMA anywhere.

Device side (all FLOPs): per chunk, one TensorE matmul projects 128 edges;
VectorE computes e = as + ad + mask, leaky-relu, and accumulates
msg += h * exp(e) and den += exp(e) along the free axis
(alpha = exp(e)/sum exp(e) folded as  out = (sum exp(e)*h_src) / sum exp(e);
masked/padded slots contribute exp(-1e30) = 0).  Layer 2 (ball only) runs
fully on-chip with one-hot matmuls against the SBUF-resident projection.

The same program is replicated SPMD on all 8 NeuronCores (the cone is tiny, so
replication beats sharding + collectives); core 0's output is returned.
"""

import numpy as np

P = 128
_CACHE = {}


def _ceil(a, b):
    return -(-a // b)


def _pad_rows(a, n, fill=0):
    out = np.full((n,) + a.shape[1:], fill, a.dtype)
    out[: len(a)] = a
    return out


class _Packer:
    """Pack many small [p, w] operands into one [128, W] array, column-wise."""

    def __init__(self, dtype=np.float32):
        self.cols = []
        self.pos = 0
        self.slots = {}
        self.dtype = dtype

    def add(self, name, arr):
        p, w = arr.shape
        full = np.zeros((P, w), self.dtype)
        full[:p] = arr
        self.cols.append(full)
        self.slots[name] = (self.pos, self.pos + w)
        self.pos += w

    def finish(self):
        return np.ascontiguousarray(np.concatenate(self.cols, axis=1))


NEG = np.float32(-1e30)


def _host_preprocess(inputs):
    x = np.asarray(inputs["x"], dtype=np.float32)
    ei = np.asarray(inputs["edge_index"]).astype(np.int64)
    N = x.shape[0]
    F = x.shape[1]
    ball = N - 1
    src, dst = ei[0], ei[1]

    # ---- layer-2 cone: edges into the ball (+ self loop) --------------------
    sel2 = dst == ball
    e2s = np.concatenate([src[sel2], [ball]])
    uniq = np.unique(e2s)
    S2 = np.concatenate([[ball], uniq[uniq != ball]]).astype(np.int64)
    m2 = len(S2)
    assert m2 <= 127, f"ball in-neighbourhood too large for one dst block: {m2}"

    # ---- layer-1 edge grid: [partition, chunk] ------------------------------
    in_S2 = np.zeros(N, dtype=bool)
    in_S2[S2] = True
    sel1 = in_S2[dst]
    l1s, l1d = src[sel1], dst[sel1]  # self loops handled separately

    # per-destination source lists (excluding the self loop)
    loc2 = np.full(N, -1, dtype=np.int64)
    loc2[S2] = np.arange(m2)
    by_dst = [[] for _ in range(m2)]
    for s, d in zip(l1s, loc2[l1d]):
        by_dst[d].append(s)

    # choose K (chunks) so all partition groups fit in 128 partitions
    K = 2
    while sum(max(1, _ceil(len(g), K - 1)) for g in by_dst) > P:
        K += 1
    K = max(K, 3)

    grid_src = np.zeros((P, K), dtype=np.int64)  # source node per slot
    grid_valid = np.zeros((P, K), dtype=bool)
    slotmap = np.full(P, P - 1, dtype=np.int64)  # partition -> dst slot
    p = 0
    for sidx in range(m2):
        g = by_dst[sidx]
        v = S2[sidx]
        nparts = max(1, _ceil(len(g), K - 1))
        for gi in range(nparts):
            grid_src[p, 0] = v  # self loop (duplicates masked)
            grid_valid[p, 0] = gi == 0
            chunk_edges = g[gi * (K - 1) : (gi + 1) * (K - 1)]
            for j, s in enumerate(chunk_edges):
                grid_src[p, 1 + j] = s
                grid_valid[p, 1 + j] = True
            slotmap[p] = sidx
            p += 1
    assert p <= P

    # xT: [F, K*128] with column k*128+q = x[grid_src[q, k]] (bf16).
    # Chunk-0 columns keep their features even when masked: secondary
    # partitions read a_dst[dst] from their (duplicate) self-loop row.
    zero_slots = ~grid_valid
    zero_slots[:, 0] = False
    xg = x[grid_src.T.reshape(-1)]  # [K*128, F]
    xg[zero_slots.T.reshape(-1)] = 0
    import ml_dtypes

    xT = np.ascontiguousarray(xg.T.astype(ml_dtypes.bfloat16))  # [F, K*128]

    admask = np.where(grid_valid, np.float32(0), NEG).astype(np.float32)  # [P,K]
    pmapcol = slotmap[:, None].astype(np.float32)  # [P,1]

    # ---- layer-2 index rows -------------------------------------------------
    s2_loc = loc2[e2s]  # all < m2
    n2 = len(s2_loc)
    T2 = _ceil(n2, P)
    assert T2 == 1, f"layer-2 edge count exceeds one tile: {n2}"
    n2p = T2 * P

    def row(a, n_pad, fill, dt):
        return _pad_rows(a.astype(dt), n_pad, fill)[None, :]

    # ---- dense operands -----------------------------------------------------
    W1 = np.asarray(inputs["W1"], np.float32)  # [F, 4*64]
    a_src1 = np.asarray(inputs["a_src1"], np.float32)  # [4, 64]
    a_dst1 = np.asarray(inputs["a_dst1"], np.float32)
    H1, C = a_src1.shape
    D1 = H1 * C
    ablk = np.zeros((D1, 2 * H1), np.float32)  # [256, 8] = [Ad | As]
    for h in range(H1):
        ablk[h * C : (h + 1) * C, h] = a_dst1[h]
        ablk[h * C : (h + 1) * C, H1 + h] = a_src1[h]

    W2 = np.asarray(inputs["W2"], np.float32)  # [256, 64]
    a2 = np.stack(
        [np.asarray(inputs["a_dst2"], np.float32)[0],
         np.asarray(inputs["a_src2"], np.float32)[0]],
        axis=1,
    )  # [64, 2] = [a_dst | a_src]

    pkw = _Packer()
    pkw.add("w1", W1)
    W1T = np.ascontiguousarray(W1.T)
    for k in range(D1 // P):
        pkw.add(f"w1T{k}", W1T[k * P : (k + 1) * P])
        pkw.add(f"ablk{k}", ablk[k * P : (k + 1) * P])

    pka = _Packer()
    pka.add("admask", admask)
    pka.add("pmapcol", pmapcol)

    pkb = _Packer()
    for k in range(D1 // P):
        pkb.add(f"w2_{k}", W2[k * P : (k + 1) * P])
    pkb.add("b1bc", np.broadcast_to(np.asarray(inputs["b1"], np.float32), (P, D1)))
    pkb.add("w2T", np.ascontiguousarray(W2.T))
    pkb.add("a2", a2)
    pkb.add("b2col", np.asarray(inputs["b2"], np.float32)[:, None])
    pkb.add("fc1w", np.ascontiguousarray(np.asarray(inputs["fc1_w"], np.float32)))
    pkb.add("fc1b", np.asarray(inputs["fc1_b"], np.float32)[:, None])
    pkb.add("fc2w", np.ascontiguousarray(np.asarray(inputs["fc2_w"], np.float32)))
    pkb.add("fc2b", np.asarray(inputs["fc2_b"], np.float32)[:, None])
    pkb.add("dstrel2", np.ascontiguousarray(
        _pad_rows(np.zeros(n2, np.float32), n2p, 1)[:, None]))
    pkb.add("l2rep", np.broadcast_to(np.concatenate(
        [row(s2_loc, n2p, 0, np.float32),
         row(np.zeros(n2), n2p, 1, np.float32)], axis=1), (P, 2 * n2p)))

    feed = {"xT": xT, "packw": pkw.finish(), "packa": pka.finish(),
            "packb": pkb.finish()}
    dims = dict(
        F=F, H1=H1, C=C, K=K, m2=m2, T2=T2,
        slots_w=tuple(sorted(pkw.slots.items())),
        slots_a=tuple(sorted(pka.slots.items())),
        slots_b=tuple(sorted(pkb.slots.items())),
    )
    return feed, dims


def _build(dims):
    from concourse import bacc, bass, mybir, tile
    from concourse.masks import make_identity

    F = dims["F"]          # 128 input features
    H1 = dims["H1"]        # 4 heads, layer 1
    C = dims["C"]          # 64 channels per head
    D1 = H1 * C            # 256
    G1W = 2 * H1 + D1      # 264 = [ad(4) | as(4) | h(256)]
    G2W = 2 + C            # 66  = [ad2 | as2 | h2p]
    K = dims["K"]          # layer-1 chunks (edge slots per partition)
    KCH = D1 // P          # 2 contraction chunks over 256
    slots_w = dict(dims["slots_w"])
    slots_a = dict(dims["slots_a"])
    slots_b = dict(dims["slots_b"])
    WW = max(b for _, b in slots_w.values())
    WA = max(b for _, b in slots_a.values())
    WB = max(b for _, b in slots_b.values())
    f32 = mybir.dt.float32
    bf16 = mybir.dt.bfloat16

    nc = bacc.Bacc("TRN2", target_bir_lowering=False, debug=False)

    xT_d = nc.declare_dram_parameter("xT", [F, K * P], bf16, isOutput=False)
    pw_d = nc.declare_dram_parameter("packw", [P, WW], f32, isOutput=False)
    pa_d = nc.declare_dram_parameter("packa", [P, WA], f32, isOutput=False)
    pb_d = nc.declare_dram_parameter("packb", [P, WB], f32, isOutput=False)
    out_d = nc.declare_dram_parameter("out", [2, 1], f32, isOutput=True)

    EQ = mybir.AluOpType.is_equal
    MAX = mybir.AluOpType.max
    ADD = mybir.AluOpType.add
    MUL = mybir.AluOpType.mult
    Copy = mybir.ActivationFunctionType.Copy
    Exp = mybir.ActivationFunctionType.Exp
    Relu = mybir.ActivationFunctionType.Relu

    with tile.TileContext(nc) as tc:
        with (
            tc.tile_pool(name="const", bufs=1) as cp,
            tc.tile_pool(name="work", bufs=3) as wp,
            tc.tile_pool(name="fin", bufs=1) as fp,
            tc.tile_pool(name="psum", bufs=2, space="PSUM") as pp,
            tc.tile_pool(name="pgp", bufs=3, space="PSUM") as pgp,
            tc.tile_pool(name="acc", bufs=1, space="PSUM") as ap_,
        ):
            # ---------------- inputs into SBUF -----------------------------
            pkw_s = cp.tile([P, WW], f32)
            nc.sync.dma_start(pkw_s[:], pw_d[:])
            pka_s = cp.tile([P, WA], f32)
            nc.sync.dma_start(pka_s[:], pa_d[:])
            xT_s = cp.tile([F, K * P], bf16)
            nc.sync.dma_start(xT_s[:], xT_d[:])
            pkb_s = cp.tile([P, WB], f32)
            nc.gpsimd.dma_start(pkb_s[:], pb_d[:])

            def fsl(name, rows=P):
                if name in slots_w:
                    a, b = slots_w[name]
                    return pkw_s[:rows, a:b]
                if name in slots_a:
                    a, b = slots_a[name]
                    return pka_s[:rows, a:b]
                a, b = slots_b[name]
                return pkb_s[:rows, a:b]

            ident = cp.tile([P, P], f32)
            make_identity(nc, ident[:])
            identb = cp.tile([P, P], bf16)
            nc.gpsimd.tensor_copy(identb[:], ident[:])
            iota_f = cp.tile([P, P], f32)
            nc.gpsimd.iota(
                iota_f[:], pattern=[[1, P]], base=0, channel_multiplier=0,
                allow_small_or_imprecise_dtypes=True,
            )
            iota_c = cp.tile([P, 1], f32)
            nc.gpsimd.iota(
                iota_c[:], pattern=[[0, 1]], base=0, channel_multiplier=1,
                allow_small_or_imprecise_dtypes=True,
            )

            # ---------------- W1 @ [Ad | As]  (K = 256, 2 chunks) ----------
            pwa = pp.tile([F, 2 * H1], f32, tag="mm")
            for k in range(KCH):
                nc.tensor.matmul(
                    out=pwa[:], lhsT=fsl(f"w1T{k}"), rhs=fsl(f"ablk{k}"),
                    start=(k == 0), stop=(k == KCH - 1),
                )
            rhs1 = cp.tile([F, G1W], bf16)
            nc.vector.tensor_copy(rhs1[:, : 2 * H1], pwa[:])
            nc.scalar.copy(rhs1[:, 2 * H1 :], fsl("w1"))

            # ---------------- layer-1 edge chunks --------------------------
            # chunk k: project 128 edge slots, form [h*exp(e) | exp(e)], and
            # accumulate + merge partition groups in PSUM via the one-hot
            # partition->slot matmul:  agg1 += pmap @ [msg | exp(e)]
            pmap = fp.tile([P, P], bf16)
            nc.vector.tensor_scalar(
                pmap[:], iota_f[:], fsl("pmapcol")[:, 0:1], None, EQ
            )
            agg1 = ap_.tile([P, D1 + H1], f32, tag="agg1")
            ad_part = fp.tile([P, H1], f32)
            admix = fp.tile([P, K * H1], f32)
            for k in range(K):
                pg = pgp.tile([P, G1W], f32, tag="pg")
                nc.tensor.matmul(
                    out=pg[:], lhsT=xT_s[:, k * P : (k + 1) * P],
                    rhs=rhs1[:], start=True, stop=True,
                )
                if k == 0:
                    # a_dst per partition from the self-loop rows, then fold
                    # in the validity mask for every chunk at once
                    nc.vector.tensor_copy(ad_part[:], pg[:, :H1])
                    nc.vector.tensor_tensor(
                        out=admix[:].rearrange("p (k h) -> p k h", h=H1),
                        in0=ad_part[:].rearrange("p (o h) -> p o h", o=1)
                        .to_broadcast([P, K, H1]),
                        in1=fsl("admask")[:].rearrange("p (k o) -> p k o", o=1)
                        .to_broadcast([P, K, H1]),
                        op=ADD,
                    )
                e = wp.tile([P, H1], f32, tag="e")
                nc.vector.tensor_tensor(
                    out=e[:], in0=pg[:, H1 : 2 * H1],
                    in1=admix[:, k * H1 : (k + 1) * H1], op=ADD,
                )
                es = wp.tile([P, H1], f32, tag="es")
                nc.gpsimd.tensor_scalar_mul(es[:], e[:], 0.2)
                el = wp.tile([P, H1], f32, tag="el")
                nc.vector.tensor_tensor(out=el[:], in0=e[:], in1=es[:], op=MAX)
                pe = wp.tile([P, H1], f32, tag="pe")
                nc.scalar.activation(pe[:], el[:], Exp)
                msg = wp.tile([P, D1 + H1], bf16, tag="msg")
                nc.vector.tensor_tensor(
                    out=msg[:, :D1].rearrange("p (h c) -> p h c", c=C),
                    in0=pg[:, 2 * H1 :].rearrange("p (h c) -> p h c", c=C),
                    in1=pe[:].rearrange("p (h o) -> p h o", o=1).to_broadcast(
                        [P, H1, C]
                    ),
                    op=MUL,
                )
                nc.gpsimd.tensor_copy(msg[:, D1:], pe[:])
                nc.tensor.matmul(
                    out=agg1[:], lhsT=pmap[:], rhs=msg[:],
                    start=(k == 0), stop=(k == K - 1),
                )

            # layer-2 rhs [W2A2 | W2] per K-chunk + one-hots (no h1r dep)
            rhs2 = []
            for k in range(KCH):
                pwa2 = pp.tile([P, 2], f32, tag="mm")
                nc.tensor.matmul(
                    out=pwa2[:], lhsT=fsl("w2T", C)[:, k * P : (k + 1) * P],
                    rhs=fsl("a2", C), start=True, stop=True,
                )
                rhs2_k = cp.tile([P, G2W], bf16, name=f"rhs2_{k}")
                nc.vector.tensor_copy(rhs2_k[:, :2], pwa2[:])
                nc.scalar.copy(rhs2_k[:, 2:], fsl(f"w2_{k}"))
                rhs2.append(rhs2_k)
            # layer-2 one-hots from the replicated [src2 | dstrel2] rows
            l2r = fsl("l2rep")
            st2 = fp.tile([P, P], bf16)
            nc.vector.tensor_scalar(st2[:], l2r[:, :P], iota_c[:, 0:1], None, EQ)
            em2t = fp.tile([P, P], f32)
            nc.vector.tensor_scalar(em2t[:], l2r[:, P:], iota_c[:, 0:1], None, EQ)

            # ---------------- layer-1 finalize: h1r = relu(num/den + b1) ---
            den1 = fp.tile([P, H1], f32)
            nc.vector.tensor_scalar_add(den1[:], agg1[:, D1:], 1e-16)
            rec1 = fp.tile([P, H1], f32)
            nc.vector.reciprocal(rec1[:], den1[:])
            h1t = fp.tile([P, D1], bf16)
            nc.vector.tensor_tensor(
                out=h1t[:].rearrange("p (h c) -> p h c", c=C),
                in0=agg1[:, :D1].rearrange("p (h c) -> p h c", c=C),
                in1=rec1[:].rearrange("p (h o) -> p h o", o=1).to_broadcast(
                    [P, H1, C]
                ),
                op=MUL,
            )
            h1b = fp.tile([P, D1], bf16)
            nc.vector.tensor_add(h1b[:], h1t[:], fsl("b1bc"))
            h1r = fp.tile([P, D1], bf16)
            nc.scalar.activation(h1r[:], h1b[:], Relu)

            # ---------------- layer-2 projection: G2 = [ad2|as2|h2p] -------
            pg2 = ap_.tile([P, G2W], f32, tag="pg2")
            for k in range(KCH):
                ptr = pp.tile([P, P], bf16, tag="mm")
                nc.tensor.transpose(
                    out=ptr[:], in_=h1r[:, k * P : (k + 1) * P],
                    identity=identb[:],
                )
                h1rT_k = wp.tile([P, P], bf16, tag=f"h1rTk{k}")
                nc.vector.tensor_copy(h1rT_k[:], ptr[:])
                nc.tensor.matmul(
                    out=pg2[:], lhsT=h1rT_k[:], rhs=rhs2[k][:],
                    start=(k == 0), stop=(k == KCH - 1),
                )
            g2sb = fp.tile([P, G2W], bf16)
            nc.scalar.copy(g2sb[:], pg2[:])
            g2ad = fp.tile([P, 1], f32)
            nc.vector.tensor_copy(g2ad[:], pg2[:, 0:1])

            # ------- layer-2 edge aggregation: fully on-chip (ball only) ---
            gs2_p = pp.tile([P, G2W], f32, tag="mm")
            nc.tensor.matmul(out=gs2_p[:], lhsT=st2[:], rhs=g2sb[:],
                             start=True, stop=False, skip_group_check=True)
            # accumulate the a_dst[ball] expansion straight onto the as2
            # column: gs2_p[:, 1] becomes e2 = as2[src] + ad2[dst]
            nc.tensor.matmul(out=gs2_p[:, 1:2], lhsT=em2t[:], rhs=g2ad[:],
                             start=False, stop=True, skip_group_check=True)
            es2 = fp.tile([P, 1], f32)
            nc.vector.tensor_scalar_mul(es2[:], gs2_p[:, 1:2], 0.2)
            el2 = fp.tile([P, 1], f32)
            nc.vector.tensor_tensor(
                out=el2[:], in0=gs2_p[:, 1:2], in1=es2[:], op=MAX
            )
            pe2 = fp.tile([P, 1], f32)
            nc.scalar.activation(pe2[:], el2[:], Exp)
            rhs2t = fp.tile([P, C + 1], bf16)
            nc.vector.tensor_tensor(
                out=rhs2t[:, :C], in0=gs2_p[:, 2:],
                in1=pe2[:].to_broadcast([P, C]), op=MUL,
            )
            nc.scalar.activation(rhs2t[:, C:], pe2[:], Copy)
            em2 = fp.tile([P, P], bf16)
            nc.vector.tensor_scalar(
                em2[:], iota_f[:], fsl("dstrel2")[:, 0:1], None, EQ
            )
            agg2 = ap_.tile([P, C + 1], f32, tag="agg2")
            nc.tensor.matmul(out=agg2[:], lhsT=em2[:], rhs=rhs2t[:],
                             start=True, stop=True)

            # ---------------- ball finalize + MLP --------------------------
            den2 = fp.tile([1, 1], f32)
            nc.vector.tensor_scalar_add(den2[:], agg2[0:1, C : C + 1], 1e-16)
            rec2 = fp.tile([1, 1], f32)
            nc.vector.reciprocal(rec2[:], den2[:])
            bf = fp.tile([1, C], f32)
            nc.scalar.activation(bf[:], agg2[0:1, :C], Copy, scale=rec2[:, 0:1])
            ptb = pp.tile([C, 1], f32, tag="mm")
            nc.tensor.transpose(out=ptb[:], in_=bf[:], identity=ident[0:1, 0:1])
            bfr = fp.tile([C, 1], f32)
            nc.scalar.activation(bfr[:], ptb[:], Relu, bias=fsl("b2col", C))

            pz = pp.tile([C // 2, 1], f32, tag="mm")
            nc.tensor.matmul(out=pz[:], lhsT=fsl("fc1w", C), rhs=bfr[:],
                             start=True, stop=True)
            zr = fp.tile([C // 2, 1], f32)
            nc.scalar.activation(zr[:], pz[:], Relu, bias=fsl("fc1b", C // 2))

            po = pp.tile([2, 1], f32, tag="mm")
            nc.tensor.matmul(out=po[:], lhsT=fsl("fc2w", C // 2), rhs=zr[:],
                             start=True, stop=True)
            osb = fp.tile([2, 1], f32)
            nc.vector.tensor_add(osb[:], po[:], fsl("fc2b", 2))
            nc.sync.dma_start(out_d[:], osb[:])

    nc.compile()
    return nc


def kernel(**inputs):
    from concourse.bass_utils import run_bass_kernel_spmd

    feed, dims = _host_preprocess(inputs)
    key = (dims["K"], dims["m2"], dims["T2"])
    if key not in _CACHE:
        _CACHE[key] = _build(dims)
    nc = _CACHE[key]

    n_cores = 8
    in_maps = [dict(feed) for _ in range(n_cores)]
    res = run_bass_kernel_spmd(nc, in_maps, core_ids=list(range(n_cores)))
    out = np.asarray(res.results[0]["out"], dtype=np.float32).reshape(2)
    return out
